# revision 33
# baseline (speedup 1.0000x reference)
"""Bass/Tile Trainium2 kernel for nn_Net_4698694222696.

PANConv (cubic path-integral filter) + PANPooling (top-k) + GCNConv + sum-pool
+ linear head + log_softmax, data-parallel over the graph dimension:
64 graphs -> 8 NeuronCores x 8 graphs/core (no collectives needed).

Algorithm notes (per graph, N=512 nodes, 4 row-chunks of 128):
  M = c0*I + c1*A + c2*A^2 + c3*A^3 via two bf16 PE chains (exact: A is 0/1
  and D = c3*A^2 + c2*A + c1*I is small-integer*2^-4 for pan_weight=0.5;
  PSUM accumulates fp32): A2 = A@A ; D assembled in the drain ; M = A@D + c0*I.
  Mn = diag(d) M diag(d) never materialized - d folded into matmul operands.
  top-k via exact comparison-counting rank (ties broken by index, matching
  jax.lax.top_k); pooled-feature gather via 0/1 selection matmul; pooled
  adjacency via GPSIMD indirect_copy column gather + one selection matmul.
  rsqrt via uint32 bit-trick seed + 2 Newton iterations on DVE (ACT sqrt is
  imprecise; keeps ACT in one table set so no per-graph table reloads).
  Graph loop is software-pipelined (s1 chains | s1t score | s2a rank | s2b
  gather, lag 2) with a batched GCN epilogue over graph halves.
"""

import numpy as np

G_TOT, N, F_IN, HID, K, CLS = 64, 512, 7, 64, 128, 2
NCORES = 8
NG = G_TOT // NCORES  # graphs per core
P = 128
T = N // P  # 4 row-chunks

_CACHE = {}


def _blk(t):
    return slice(t * P, (t + 1) * P)


def _rsqrt(nc, pool, x, magic_u, ones_u, Alu, f32, u32, name):
    """y = x**-0.5 elementwise for an SBUF tile x of shape [P, w]."""
    w = x.shape[-1]
    yi = pool.tile(list(x.shape), u32, name=name + "_i")
    # seed bits = (2*0x5f3759df - bits(x)) >> 1  ~  0x5f3759df - (bits(x)>>1)
    nc.vector.tensor_tensor(out=yi, in0=magic_u[:, :w], in1=x.bitcast(u32), op=Alu.subtract)
    yi2 = pool.tile(list(x.shape), u32, name=name + "_i2")
    nc.vector.tensor_tensor(out=yi2, in0=yi, in1=ones_u[:, :w], op=Alu.logical_shift_right)
    y = yi2.bitcast(f32)
    t = pool.tile(list(x.shape), f32, name=name + "_t")
    y2 = pool.tile(list(x.shape), f32, name=name + "_y2")
    cur, nxt = y, y2
    for _ in range(2):
        nc.vector.tensor_tensor(out=t, in0=cur, in1=cur, op=Alu.mult)
        nc.vector.tensor_tensor(out=t, in0=t, in1=x, op=Alu.mult)
        nc.vector.tensor_scalar(out=t, in0=t, scalar1=-0.5, scalar2=1.5, op0=Alu.mult, op1=Alu.add)
        nc.vector.tensor_tensor(out=nxt, in0=cur, in1=t, op=Alu.mult)
        cur, nxt = nxt, cur
    return cur


def build_program():
    """Build the single-core SPMD Bass program (same NEFF on all 8 cores)."""
    from contextlib import ExitStack

    import concourse.bass as bass
    import concourse.bacc as bacc
    import concourse.mybir as mybir
    import concourse.tile as tile
    from concourse.masks import make_identity

    f32 = mybir.dt.float32
    bf16 = mybir.dt.bfloat16
    u32 = mybir.dt.uint32
    u16 = mybir.dt.uint16
    Alu = mybir.AluOpType
    Act = mybir.ActivationFunctionType
    X = mybir.AxisListType.X

    nc = bacc.Bacc("TRN2", target_bir_lowering=False, debug=False, num_devices=NCORES)

    # ---- per-core DRAM I/O ----
    adj_d = nc.dram_tensor("adj", [NG, N, N], bf16, kind="ExternalInput")
    xt_d = nc.dram_tensor("xt", [NG, F_IN, N], f32, kind="ExternalInput")  # x^T per graph
    w1_d = nc.dram_tensor("w1", [F_IN, HID], f32, kind="ExternalInput")
    gw_d = nc.dram_tensor("gcnw", [HID, HID], f32, kind="ExternalInput")
    lw_d = nc.dram_tensor("linw", [HID, CLS], f32, kind="ExternalInput")
    lb_d = nc.dram_tensor("linb", [NG, CLS], f32, kind="ExternalInput")
    b1_d = nc.dram_tensor("b1b", [P, HID], f32, kind="ExternalInput")
    pb_d = nc.dram_tensor("pb", [P, HID], f32, kind="ExternalInput")
    bg_d = nc.dram_tensor("bgb", [P, HID], f32, kind="ExternalInput")
    io_d = nc.dram_tensor("iota", [P, N], f32, kind="ExternalInput")
    px_d = nc.dram_tensor("pidx", [P, T], f32, kind="ExternalInput")
    cv_d = nc.dram_tensor("cvec", [P, 4], f32, kind="ExternalInput")
    bt_d = nc.dram_tensor("betab", [P, 2], f32, kind="ExternalInput")
    mg_d = nc.dram_tensor("magic", [P, NG], u32, kind="ExternalInput")
    wr_d = nc.dram_tensor("wrapidx", [P, P], f32, kind="ExternalInput")
    out_d = nc.dram_tensor("out", [NG, CLS], f32, kind="ExternalOutput")
    # internal DRAM scratch for the score row-broadcast round trip
    srow_d = nc.dram_tensor("srow", [NG, N], f32)
    idx_d = nc.dram_tensor("idxscr", [NG, P], f32)

    adj_ap = adj_d.ap()
    xt_ap = xt_d.ap()

    with tile.TileContext(nc) as tc, ExitStack() as ctx:
        consts = ctx.enter_context(tc.tile_pool(name="consts", bufs=1))
        pa = ctx.enter_context(tc.tile_pool(name="pa", bufs=3))
        pbd = ctx.enter_context(tc.tile_pool(name="pbd", bufs=2))
        pmm = ctx.enter_context(tc.tile_pool(name="pmm", bufs=3))
        psm = ctx.enter_context(tc.tile_pool(name="psm", bufs=2))
        pwide = ctx.enter_context(tc.tile_pool(name="pwide", bufs=2))
        ppb = ctx.enter_context(tc.tile_pool(name="ppb", bufs=2, space="PSUM"))
        pp65 = ctx.enter_context(tc.tile_pool(name="pp65", bufs=2, space="PSUM"))
        pps = ctx.enter_context(tc.tile_pool(name="pps", bufs=3, space="PSUM"))
        pmp = ctx.enter_context(tc.tile_pool(name="pmp", bufs=NG))

        # ---- prefetch graph 0 before the constant DMAs ----
        A0 = pa.tile([P, T, N], bf16, name="A")
        nc.sync.dma_start(A0, adj_ap[0].rearrange("(t p) j -> p t j", p=P))
        xt0 = psm.tile([F_IN, N], f32, name="xtt")
        nc.sync.dma_start(xt0, xt_ap[0])

        # ---- session constants ----
        io_sb = consts.tile([P, N], f32)
        nc.sync.dma_start(io_sb, io_d.ap())
        px_sb = consts.tile([P, T], f32)
        nc.sync.dma_start(px_sb, px_d.ap())
        cv_sb = consts.tile([P, 4], f32)
        nc.sync.dma_start(cv_sb, cv_d.ap())
        bt_sb = consts.tile([P, 2], f32)
        nc.sync.dma_start(bt_sb, bt_d.ap())
        mg_sb = consts.tile([P, NG], u32)
        nc.sync.dma_start(mg_sb, mg_d.ap())
        wr_sb = consts.tile([P, P], f32)
        nc.sync.dma_start(wr_sb, wr_d.ap())
        b1_sb = consts.tile([P, HID], f32)
        nc.sync.dma_start(b1_sb, b1_d.ap())
        pb_sb = consts.tile([P, HID], f32)
        nc.sync.dma_start(pb_sb, pb_d.ap())
        bg_sb = consts.tile([P, HID], f32)
        nc.sync.dma_start(bg_sb, bg_d.ap())
        w1_sb = consts.tile([F_IN, HID], f32)
        nc.sync.dma_start(w1_sb, w1_d.ap())
        gw_sb = consts.tile([HID, HID], f32)
        nc.sync.dma_start(gw_sb, gw_d.ap())
        lw_sb = consts.tile([HID, CLS], f32)
        nc.sync.dma_start(lw_sb, lw_d.ap())
        lb_sb = consts.tile([NG, CLS], f32)
        nc.sync.dma_start(lb_sb, lb_d.ap())

        ones_u = consts.tile([P, NG], u32)
        nc.vector.memset(ones_u, 1)
        ones_col = consts.tile([P, 1], f32)
        nc.vector.memset(ones_col, 1.0)
        onec_bf = consts.tile([P, 1], bf16)
        nc.vector.memset(onec_bf, 1.0)

        eye_sb = consts.tile([P, P], f32)
        make_identity(nc, eye_sb)
        eye_c0 = consts.tile([P, P], f32)
        eye_c1 = consts.tile([P, P], f32)
        eye_c2 = consts.tile([P, P], f32)
        nc.vector.tensor_scalar(out=eye_c0, in0=eye_sb, scalar1=cv_sb[:, 0:1], scalar2=None, op0=Alu.mult)
        nc.vector.tensor_scalar(out=eye_c1, in0=eye_sb, scalar1=cv_sb[:, 1:2], scalar2=None, op0=Alu.mult)
        nc.vector.tensor_scalar(out=eye_c2, in0=eye_sb, scalar1=cv_sb[:, 2:3], scalar2=None, op0=Alu.mult)

        # lower-triangle masks: ltm[p, t, j] = 1 if j < 128*t + p
        ltm = consts.tile([P, T, N], f32)
        for t in range(T):
            nc.gpsimd.tensor_scalar(out=ltm[:, t, :], in0=io_sb, scalar1=px_sb[:, t : t + 1], scalar2=None, op0=Alu.is_lt)

        pooled_all = consts.tile([HID, NG], f32)
        xp_all = consts.tile([P, NG, HID], f32)
        dsel_all = consts.tile([P, NG], f32)
        dgpre_all = consts.tile([P, NG], f32)


        def prefetch(g):
            A = pa.tile([P, T, N], bf16, name="A")
            nc.sync.dma_start(A, adj_ap[g].rearrange("(t p) j -> p t j", p=P))
            xt = psm.tile([F_IN, N], f32, name="xtt")
            nc.sync.dma_start(xt, xt_ap[g])
            return A, xt

        def stage1(g, pre):
            """Filter chains + degree for graph g."""
            A, xt = pre

            # ---- A2 = A@A ; D = c3*A2 + c2*A + c1*I (bf16, exact) ----
            D = pbd.tile([P, T, N], bf16, name="D")
            for i in range(T):
                ps = ppb.tile([P, N], f32, name="psC", tag="big")
                for k in range(T):
                    nc.tensor.matmul(ps, lhsT=A[:, k, _blk(i)], rhs=A[:, k, :], start=(k == 0), stop=(k == T - 1))
                a2t = pwide.tile([P, N], bf16, name="a2t")
                nc.scalar.activation(out=a2t, in_=ps, func=Act.Copy, scale=cv_sb[:, 3:4])
                nc.vector.scalar_tensor_tensor(out=D[:, i, :], in0=A[:, i, :], scalar=cv_sb[:, 2:3], in1=a2t, op0=Alu.mult, op1=Alu.add)
                nc.gpsimd.tensor_tensor(out=D[:, i, _blk(i)], in0=D[:, i, _blk(i)], in1=eye_c1, op=Alu.add)

            # ---- M = A@D + c0*I ; deg row-sums fused into the drains ----
            M = pmm.tile([P, T, N], f32, name="M")
            degr = psm.tile([P, T], f32, name="degr")
            for i in range(T):
                ps = ppb.tile([P, N], f32, name="psM", tag="big")
                for k in range(T):
                    nc.tensor.matmul(ps, lhsT=A[:, k, _blk(i)], rhs=D[:, k, :], start=(k == 0), stop=(k == T - 1))
                if i == 0:
                    nc.vector.tensor_scalar(out=M[:, i, :], in0=ps, scalar1=0.0, scalar2=None, op0=Alu.add, op1=Alu.add, accum_out=degr[:, i : i + 1])
                else:
                    nc.scalar.activation(out=M[:, i, :], in_=ps, func=Act.Copy, accum_out=degr[:, i : i + 1])
                nc.gpsimd.tensor_tensor(out=M[:, i, _blk(i)], in0=M[:, i, _blk(i)], in1=eye_c0, op=Alu.add)

            return dict(A=A, xt=xt, M=M, degr=degr)

        def stage1t(g, st):
            """rsqrt + conv + score for graph g."""
            xt, M, degr = st["xt"], st["M"], st["degr"]
            # deg = clip(degr + c0, 1, inf);  d = deg**-0.5
            deg4 = psm.tile([P, T], f32, name="deg4")
            nc.vector.tensor_scalar(out=deg4, in0=degr, scalar1=cv_sb[:, 0:1], scalar2=1.0, op0=Alu.add, op1=Alu.max)
            d4 = _rsqrt(nc, psm, deg4, mg_sb, ones_u, Alu, f32, u32, name="d4")

            # ---- rhs65 = [ d*(x @ W1) | d ]  per row-chunk ----
            rhs65 = pwide.tile([P, T, HID + 1], f32, name="rhs65")
            for t in range(T):
                psx = pps.tile([P, HID], f32, name="psxw", tag="ps128")
                nc.tensor.matmul(psx, lhsT=xt[:, _blk(t)], rhs=w1_sb, start=True, stop=True)
                nc.vector.tensor_scalar(out=rhs65[:, t, 0:HID], in0=psx, scalar1=d4[:, t : t + 1], scalar2=None, op0=Alu.mult)
                nc.scalar.copy(rhs65[:, t, HID : HID + 1], d4[:, t : t + 1])

            # ---- fused: [h | M@d] = M @ rhs65 ; h = relu(d_i*(.) + b1) ----
            # hsc holds [h | score | d | node_id] as the rhs of the xv gather chain
            hsc = pwide.tile([P, T, HID + 3], f32, name="hsc")
            s1c = psm.tile([P, T], f32, name="s1c")
            s2b = psm.tile([P, T], f32, name="s2b")
            junkh = psm.tile([P, HID], f32, name="junkh")
            for i in range(T):
                ps65 = pp65.tile([P, HID + 1], f32, name="ps65", tag="p65")
                for k in range(T):
                    nc.tensor.matmul(ps65, lhsT=M[:, k, _blk(i)], rhs=rhs65[:, k, :], start=(k == 0), stop=(k == T - 1))
                hraw = psm.tile([P, HID], f32, name="hraw")
                nc.vector.scalar_tensor_tensor(out=hraw, in0=ps65[:, 0:HID], scalar=d4[:, i : i + 1], in1=b1_sb, op0=Alu.mult, op1=Alu.add)
                nc.scalar.activation(out=hsc[:, i, 0:HID], in_=hraw, func=Act.Relu)
                # s1_i = sum_h h*p ;  s2b_i = beta1 * d_i * (M@d)_i
                nc.vector.scalar_tensor_tensor(out=junkh, in0=hsc[:, i, 0:HID], scalar=1.0, in1=pb_sb, op0=Alu.mult, op1=Alu.mult, accum_out=s1c[:, i : i + 1])
                nc.vector.scalar_tensor_tensor(out=s2b[:, i : i + 1], in0=ps65[:, HID : HID + 1], scalar=d4[:, i : i + 1], in1=bt_sb[:, 1:2], op0=Alu.mult, op1=Alu.mult)
                nc.scalar.copy(hsc[:, i, HID + 1 : HID + 2], d4[:, i : i + 1])
                nc.scalar.copy(hsc[:, i, HID + 2 : HID + 3], px_sb[:, i : i + 1])

            # ---- score = tanh(beta0*s1 + s2b) ----
            z4 = psm.tile([P, T], f32, name="z4")
            nc.vector.scalar_tensor_tensor(out=z4, in0=s1c, scalar=bt_sb[:, 0:1], in1=s2b, op0=Alu.mult, op1=Alu.add)
            sc4 = psm.tile([P, T], f32, name="sc4")
            nc.scalar.activation(out=sc4, in_=z4, func=Act.Tanh)
            for i in range(T):
                nc.scalar.copy(hsc[:, i, HID : HID + 1], sc4[:, i : i + 1])

            # ---- broadcast score along free dim via DRAM round trip ----
            nc.sync.dma_start(bass.AP(srow_d, g * N, [[1, P], [P, T]]), sc4)
            scbf = pwide.tile([P, N], f32, name="scb")
            nc.sync.dma_start(scbf, bass.AP(srow_d, g * N, [[0, P], [1, N]]))
            st.update(d4=d4, hsc=hsc, sc4=sc4, scbf=scbf)
            return st

        def stage2a(g, st):
            """Rank/top-k + pooled-feature gather setup for graph g."""
            M, d4, hsc, sc4, scbf = st["M"], st["d4"], st["hsc"], st["sc4"], st["scbf"]
            # ---- exact rank: #(s_j > s_i) + #(s_j == s_i and j < i) ----
            junk2 = pwide.tile([P, N], f32, name="junk2")
            req = psm.tile([P, T], f32, name="req")
            masks = []
            for jc in range(T):
                mj = pwide.tile([P, N], bf16, name="mj%d" % jc)
                eng1 = nc.gpsimd if jc < 3 else nc.vector
                eng1.tensor_scalar(out=mj, in0=scbf, scalar1=sc4[:, jc : jc + 1], scalar2=None, op0=Alu.is_lt)
                masks.append(mj)
            for i in range(T):
                w = P * (i + 1)
                nc.vector.scalar_tensor_tensor(out=junk2[:, 0:w], in0=scbf[:, 0:w], scalar=sc4[:, i : i + 1], in1=ltm[:, i, 0:w], op0=Alu.is_equal, op1=Alu.mult, accum_out=req[:, i : i + 1])
            psRank = pps.tile([P, T], f32, name="psRank", tag="ps128")
            for ic in range(T):
                for jc in range(T):
                    nc.tensor.matmul(psRank[:, ic : ic + 1], lhsT=masks[jc][:, _blk(ic)], rhs=onec_bf, start=(jc == 0), stop=(jc == T - 1))
            rank = psm.tile([P, T], f32, name="rank")
            nc.vector.tensor_tensor(out=rank, in0=psRank, in1=req, op=Alu.add)

            # ---- selection matrix: ST[i,r] = (rank_i == r) ----
            ST = pmm.tile([P, T, K], f32, name="ST")
            for i in range(T):
                nc.vector.tensor_scalar(out=ST[:, i, :], in0=io_sb[:, 0:K], scalar1=rank[:, i : i + 1], scalar2=None, op0=Alu.is_equal)

            # ---- pooled features: [xp0 | vals | dsel | selid] = S @ hsc ----
            psxv = pp65.tile([P, HID + 3], f32, name="psxv", tag="p65")
            for i in range(T):
                nc.tensor.matmul(psxv, lhsT=ST[:, i, :], rhs=hsc[:, i, :], start=(i == 0), stop=(i == T - 1))
            vals = psm.tile([P, 1], f32, name="vals")
            nc.scalar.copy(vals, psxv[:, HID : HID + 1])
            dsel = psm.tile([P, 1], f32, name="dsel")
            nc.scalar.copy(dsel, psxv[:, HID + 1 : HID + 2])
            nc.scalar.copy(dsel_all[:, g : g + 1], psxv[:, HID + 1 : HID + 2])
            selid = psm.tile([P, 1], f32, name="selid")
            nc.scalar.copy(selid, psxv[:, HID + 2 : HID + 3])
            nc.vector.tensor_scalar(out=xp_all[:, g, :], in0=psxv[:, 0:HID], scalar1=vals, scalar2=None, op0=Alu.mult)

            # ---- selected-node index list in wrapped u16 layout via DRAM bcast ----
            nc.sync.dma_start(bass.AP(idx_d, g * P, [[1, P], [1, 1]]), selid)
            selbc = pwide.tile([P, P], f32, name="selbc")
            nc.sync.dma_start(selbc, bass.AP(idx_d, g * P, [[0, P], [1, P]]))
            selmm = pwide.tile([P, P], f32, name="selmm")
            nc.gpsimd.tensor_tensor(out=selmm, in0=selbc, in1=wr_sb, op=Alu.mult)
            idxf = psm.tile([P, 8], f32, name="idxf")
            nc.vector.tensor_reduce(out=idxf, in_=selmm.rearrange("p (s q) -> p s q", q=16), axis=X, op=Alu.max)
            idxw = psm.tile([P, 8], u16, name="idxw")
            nc.vector.tensor_copy(idxw, idxf)

            st.update(ST=ST, idxw=idxw, dsel=dsel, psxv=None)
            return st

        def stage2b(g, st):
            """Pooled adjacency Mp0 + per-graph drains for the batched tail."""
            M, ST, idxw, dsel_c, psxv = st["M"], st["ST"], st["idxw"], st["dsel"], st["psxv"]
            # ---- Mp0[r,r'] = M[sel_r, sel_r'] : free-dim gather + one matmul ----
            G2 = pmm.tile([P, T, K], f32, name="Gsb")
            for i in range(T):
                nc.gpsimd.indirect_copy(out=G2[:, i, :], data=M[:, i, :], idxs=idxw, i_know_ap_gather_is_preferred=True)
            psmp = pps.tile([P, K], f32, name="psmp", tag="ps128")
            for i in range(T):
                nc.tensor.matmul(psmp, lhsT=ST[:, i, :], rhs=G2[:, i, :], start=(i == 0), stop=(i == T - 1))
            Mp0 = pmp.tile([P, K], f32, name="Mp0")
            nc.scalar.copy(Mp0, psmp)
            # dgpre = Mp0 @ dsel
            psdg = pps.tile([P, 1], f32, name="psdg", tag="ps128")
            nc.tensor.matmul(psdg, lhsT=Mp0, rhs=dsel_c, start=True, stop=True)
            nc.scalar.copy(dgpre_all[:, g : g + 1], psdg)
            return Mp0

        def epilogue(mp0s, g0, g1):
            """Batched GCN + readout for graphs [g0, g1)."""
            NB = g1 - g0
            gs = slice(g0, g1)
            # dg = dsel*(Mp0@dsel) + 1 ; di = dg**-0.5   (batched)
            dg_all = psm.tile([P, NB], f32, name="dg_all")
            nc.vector.scalar_tensor_tensor(out=dg_all, in0=dgpre_all[:, gs], scalar=1.0, in1=dsel_all[:, gs], op0=Alu.mult, op1=Alu.mult)
            nc.vector.tensor_scalar(out=dg_all, in0=dg_all, scalar1=1.0, scalar2=None, op0=Alu.add)
            di_all = _rsqrt(nc, psm, dg_all, mg_sb, ones_u, Alu, f32, u32, name="di")
            di_bc = di_all[:, :, None].broadcast_to([P, NB, HID])
            ds_bc = dsel_all[:, gs, None].broadcast_to([P, NB, HID])
            # w = di*xp ; u = dsel*w
            w_all = psm.tile([P, NB, HID], f32, name="w_all")
            nc.vector.tensor_tensor(out=w_all, in0=xp_all[:, gs, :], in1=di_bc, op=Alu.mult)
            u_all = psm.tile([P, NB, HID], f32, name="u_all")
            nc.vector.tensor_tensor(out=u_all, in0=w_all, in1=ds_bc, op=Alu.mult)
            # z = di*(dsel*(Mp0@u) + w) per graph, batched drains
            psz = pp65.tile([P, NB, HID], f32, name="pszall", tag="pbig2", bufs=1)
            for g in range(g0, g1):
                nc.tensor.matmul(psz[:, g - g0, :], lhsT=mp0s[g], rhs=u_all[:, g - g0, :], start=True, stop=True)
            q_all = psm.tile([P, NB, HID], f32, name="q_all")
            nc.vector.tensor_tensor(out=q_all, in0=psz, in1=ds_bc, op=Alu.mult)
            nc.vector.tensor_tensor(out=q_all, in0=q_all, in1=w_all, op=Alu.add)
            g1_all = psm.tile([P, NB, HID], f32, name="g1_all")
            nc.vector.tensor_tensor(out=g1_all, in0=q_all, in1=di_bc, op=Alu.mult)
            # transpose each graph's g1: [128, 64] -> [64, 128]
            g1T_all = psm.tile([HID, NB, P], f32, name="g1T_all")
            for g in range(g0, g1):
                pst_ = pps.tile([HID, P], f32, name="psg1t", tag="ps128")
                nc.tensor.transpose(pst_, g1_all[:, g - g0, :], eye_sb)
                nc.scalar.copy(g1T_all[:, g - g0, :], pst_)
            # h2 = relu(g1 @ gcn_w + gcn_b)
            psh2 = pp65.tile([P, NB, HID], f32, name="psh2all", tag="pbig2", bufs=1)
            for g in range(g0, g1):
                nc.tensor.matmul(psh2[:, g - g0, :], lhsT=g1T_all[:, g - g0, :], rhs=gw_sb, start=True, stop=True)
            bg_bc = bg_sb[:, None, :].broadcast_to([P, NB, HID])
            h2r_all = psm.tile([P, NB, HID], f32, name="h2r_all")
            nc.vector.tensor_tensor(out=h2r_all, in0=psh2, in1=bg_bc, op=Alu.add)
            h2_all = psm.tile([P, NB, HID], f32, name="h2_all")
            nc.scalar.activation(out=h2_all, in_=h2r_all, func=Act.Relu)
            # pooled[c, g] = sum_k h2[k, g, c]
            pspool = pps.tile([HID, NB], f32, name="pspool", tag="ps128")
            for g in range(g0, g1):
                nc.tensor.matmul(pspool[:, g - g0 : g - g0 + 1], lhsT=h2_all[:, g - g0, :], rhs=ones_col, start=True, stop=True)
            nc.scalar.copy(pooled_all[:, gs], pspool)

        stash = {}
        mp0s = {}
        pre = (A0, xt0)
        for g in range(NG):
            stash[g] = stage1(g, pre)
            if g + 1 < NG:
                pre = prefetch(g + 1)
            stash[g] = stage1t(g, stash[g])
            if g >= 1:
                stash[g - 1] = stage2a(g - 1, stash[g - 1])
            if g >= 2:
                mp0s[g - 2] = stage2b(g - 2, stash.pop(g - 2))
        stash[NG - 1] = stage2a(NG - 1, stash[NG - 1])
        mp0s[NG - 2] = stage2b(NG - 2, stash.pop(NG - 2))
        epilogue(mp0s, 0, NG // 2)
        mp0s[NG - 1] = stage2b(NG - 1, stash.pop(NG - 1))
        epilogue(mp0s, NG // 2, NG)

        # ---- head: logits + log_softmax for all graphs at once ----
        pslg = pps.tile([NG, CLS], f32, name="pslg", tag="ps128")
        nc.tensor.matmul(pslg, lhsT=pooled_all, rhs=lw_sb, start=True, stop=True)
        lg = psm.tile([NG, CLS], f32, name="lg")
        nc.vector.tensor_tensor(out=lg, in0=pslg, in1=lb_sb, op=Alu.add)
        mx = psm.tile([NG, 1], f32, name="mx")
        nc.vector.tensor_reduce(out=mx, in_=lg, axis=X, op=Alu.max)
        shv = psm.tile([NG, CLS], f32, name="shv")
        nc.vector.tensor_scalar(out=shv, in0=lg, scalar1=mx, scalar2=None, op0=Alu.subtract)
        ex = psm.tile([NG, CLS], f32, name="ex")
        sm = psm.tile([NG, 1], f32, name="sm")
        nc.scalar.activation(out=ex, in_=shv, func=Act.Exp, accum_out=sm)
        ls = psm.tile([NG, 1], f32, name="ls")
        nc.scalar.activation(out=ls, in_=sm, func=Act.Ln)
        res = psm.tile([NG, CLS], f32, name="res")
        nc.vector.tensor_scalar(out=res, in0=shv, scalar1=ls, scalar2=None, op0=Alu.subtract)
        nc.sync.dma_start(out_d.ap(), res)

    nc.compile()
    return nc


def _get_program():
    if "nc" not in _CACHE:
        _CACHE["nc"] = build_program()
    return _CACHE["nc"]


def make_in_maps(inputs):
    """Host-side prep: shard graphs over cores, broadcast tiny weights."""
    import ml_dtypes

    x = np.asarray(inputs["x"], np.float32)
    adj = np.ascontiguousarray(np.asarray(inputs["adj"], np.float32).astype(ml_dtypes.bfloat16))
    pw = np.asarray(inputs["pan_weight"], np.float32)
    c = np.cumprod(pw).astype(np.float32)  # [c0, c1, c2, c3]
    w1 = np.ascontiguousarray(np.asarray(inputs["conv1_w"], np.float32))
    b1 = np.asarray(inputs["conv1_b"], np.float32)
    pv = np.asarray(inputs["p_vec"], np.float32)
    beta = np.asarray(inputs["beta"], np.float32)
    gw = np.ascontiguousarray(np.asarray(inputs["gcn_w"], np.float32))
    gb = np.asarray(inputs["gcn_b"], np.float32)
    lw = np.ascontiguousarray(np.asarray(inputs["lin_w"], np.float32))
    lb = np.asarray(inputs["lin_b"], np.float32)

    xt = np.ascontiguousarray(x.transpose(0, 2, 1))  # [G, F_IN, N]
    iota = np.tile(np.arange(N, dtype=np.float32), (P, 1))
    pidx = (np.arange(P, dtype=np.float32)[:, None] + P * np.arange(T, dtype=np.float32)[None, :])
    magic = np.full((P, NG), np.uint32(2 * 0x5F3759DF), dtype=np.uint32)
    wrap = (np.arange(P)[None, :] % 16 == np.arange(P)[:, None] % 16).astype(np.float32)

    shared = {
        "w1": w1,
        "gcnw": gw,
        "linw": lw,
        "linb": np.ascontiguousarray(np.tile(lb, (NG, 1))),
        "b1b": np.ascontiguousarray(np.tile(b1, (P, 1))),
        "pb": np.ascontiguousarray(np.tile(pv, (P, 1))),
        "bgb": np.ascontiguousarray(np.tile(gb, (P, 1))),
        "iota": iota,
        "pidx": np.ascontiguousarray(pidx),
        "cvec": np.ascontiguousarray(np.tile(c, (P, 1))),
        "betab": np.ascontiguousarray(np.tile(beta, (P, 1))),
        "magic": magic,
        "wrapidx": np.ascontiguousarray(wrap),
    }
    in_maps = []
    for ci in range(NCORES):
        sl = slice(ci * NG, (ci + 1) * NG)
        m = dict(shared)
        m["adj"] = adj[sl]
        m["xt"] = xt[sl]
        in_maps.append(m)
    return in_maps


def kernel(**inputs):
    from concourse.bass_utils import run_bass_kernel_spmd

    nc = _get_program()
    in_maps = make_in_maps(inputs)
    r = run_bass_kernel_spmd(nc, in_maps, list(range(NCORES)))
    return np.ascontiguousarray(
        np.concatenate([r.results[i]["out"] for i in range(NCORES)], axis=0)
    ).astype(np.float32)



# revision 34
# speedup vs baseline: 1.0210x; 1.0210x over previous
"""Bass/Tile Trainium2 kernel for nn_Net_4698694222696.

PANConv (cubic path-integral filter) + PANPooling (top-k) + GCNConv + sum-pool
+ linear head + log_softmax, data-parallel over the graph dimension:
64 graphs -> 8 NeuronCores x 8 graphs/core (no collectives needed).

Algorithm notes (per graph, N=512 nodes, 4 row-chunks of 128):
  M = c0*I + c1*A + c2*A^2 + c3*A^3 via two bf16 PE chains (exact: A is 0/1
  and D = c3*A^2 + c2*A + c1*I is small-integer*2^-4 for pan_weight=0.5;
  PSUM accumulates fp32): A2 = A@A ; D assembled in the drain ; M = A@D + c0*I.
  Mn = diag(d) M diag(d) never materialized - d folded into matmul operands.
  top-k via exact comparison-counting rank (ties broken by index, matching
  jax.lax.top_k); pooled-feature gather via 0/1 selection matmul; pooled
  adjacency via GPSIMD indirect_copy column gather + one selection matmul.
  rsqrt via uint32 bit-trick seed + 2 Newton iterations on DVE (ACT sqrt is
  imprecise; keeps ACT in one table set so no per-graph table reloads).
  Graph loop is software-pipelined (s1 chains | s1t score | s2a rank | s2b
  gather, lag 2) with a batched GCN epilogue over graph halves.
"""

import numpy as np

G_TOT, N, F_IN, HID, K, CLS = 64, 512, 7, 64, 128, 2
NCORES = 8
NG = G_TOT // NCORES  # graphs per core
P = 128
T = N // P  # 4 row-chunks

_CACHE = {}


def _blk(t):
    return slice(t * P, (t + 1) * P)


def _rsqrt(nc, pool, x, magic_u, ones_u, Alu, f32, u32, name):
    """y = x**-0.5 elementwise for an SBUF tile x of shape [P, w]."""
    w = x.shape[-1]
    yi = pool.tile(list(x.shape), u32, name=name + "_i")
    # seed bits = (2*0x5f3759df - bits(x)) >> 1  ~  0x5f3759df - (bits(x)>>1)
    nc.vector.tensor_tensor(out=yi, in0=magic_u[:, :w], in1=x.bitcast(u32), op=Alu.subtract)
    yi2 = pool.tile(list(x.shape), u32, name=name + "_i2")
    nc.vector.tensor_tensor(out=yi2, in0=yi, in1=ones_u[:, :w], op=Alu.logical_shift_right)
    y = yi2.bitcast(f32)
    t = pool.tile(list(x.shape), f32, name=name + "_t")
    y2 = pool.tile(list(x.shape), f32, name=name + "_y2")
    cur, nxt = y, y2
    for _ in range(2):
        nc.vector.tensor_tensor(out=t, in0=cur, in1=cur, op=Alu.mult)
        nc.vector.tensor_tensor(out=t, in0=t, in1=x, op=Alu.mult)
        nc.vector.tensor_scalar(out=t, in0=t, scalar1=-0.5, scalar2=1.5, op0=Alu.mult, op1=Alu.add)
        nc.vector.tensor_tensor(out=nxt, in0=cur, in1=t, op=Alu.mult)
        cur, nxt = nxt, cur
    return cur


def build_program():
    """Build the single-core SPMD Bass program (same NEFF on all 8 cores)."""
    from contextlib import ExitStack

    import concourse.bass as bass
    import concourse.bacc as bacc
    import concourse.mybir as mybir
    import concourse.tile as tile
    from concourse.masks import make_identity

    f32 = mybir.dt.float32
    bf16 = mybir.dt.bfloat16
    u32 = mybir.dt.uint32
    u16 = mybir.dt.uint16
    Alu = mybir.AluOpType
    Act = mybir.ActivationFunctionType
    X = mybir.AxisListType.X

    nc = bacc.Bacc("TRN2", target_bir_lowering=False, debug=False, num_devices=NCORES)

    # ---- per-core DRAM I/O ----
    adj_d = nc.dram_tensor("adj", [NG, N, N], bf16, kind="ExternalInput")
    xt_d = nc.dram_tensor("xt", [NG, F_IN, N], f32, kind="ExternalInput")  # x^T per graph
    w1_d = nc.dram_tensor("w1", [F_IN, HID], f32, kind="ExternalInput")
    gw_d = nc.dram_tensor("gcnw", [HID, HID], f32, kind="ExternalInput")
    lw_d = nc.dram_tensor("linw", [HID, CLS], f32, kind="ExternalInput")
    lb_d = nc.dram_tensor("linb", [NG, CLS], f32, kind="ExternalInput")
    b1_d = nc.dram_tensor("b1b", [P, HID], f32, kind="ExternalInput")
    pb_d = nc.dram_tensor("pb", [P, HID], f32, kind="ExternalInput")
    bg_d = nc.dram_tensor("bgb", [P, HID], f32, kind="ExternalInput")
    io_d = nc.dram_tensor("iota", [P, N], f32, kind="ExternalInput")
    px_d = nc.dram_tensor("pidx", [P, T], f32, kind="ExternalInput")
    cv_d = nc.dram_tensor("cvec", [P, 4], f32, kind="ExternalInput")
    bt_d = nc.dram_tensor("betab", [P, 2], f32, kind="ExternalInput")
    mg_d = nc.dram_tensor("magic", [P, NG], u32, kind="ExternalInput")
    wr_d = nc.dram_tensor("wrapidx", [P, P], f32, kind="ExternalInput")
    out_d = nc.dram_tensor("out", [NG, CLS], f32, kind="ExternalOutput")
    # internal DRAM scratch for the score row-broadcast round trip
    srow_d = nc.dram_tensor("srow", [NG, N], f32)
    idx_d = nc.dram_tensor("idxscr", [NG, P], f32)

    adj_ap = adj_d.ap()
    xt_ap = xt_d.ap()

    with tile.TileContext(nc) as tc, ExitStack() as ctx:
        consts = ctx.enter_context(tc.tile_pool(name="consts", bufs=1))
        pa = ctx.enter_context(tc.tile_pool(name="pa", bufs=3))
        pbd = ctx.enter_context(tc.tile_pool(name="pbd", bufs=2))
        pmm = ctx.enter_context(tc.tile_pool(name="pmm", bufs=3))
        psm = ctx.enter_context(tc.tile_pool(name="psm", bufs=2))
        pwide = ctx.enter_context(tc.tile_pool(name="pwide", bufs=2))
        ppb = ctx.enter_context(tc.tile_pool(name="ppb", bufs=2, space="PSUM"))
        pp65 = ctx.enter_context(tc.tile_pool(name="pp65", bufs=2, space="PSUM"))
        pps = ctx.enter_context(tc.tile_pool(name="pps", bufs=3, space="PSUM"))
        pmp = ctx.enter_context(tc.tile_pool(name="pmp", bufs=NG))

        # ---- prefetch graph 0 before the constant DMAs ----
        A0 = pa.tile([P, T, N], bf16, name="A")
        nc.sync.dma_start(A0, adj_ap[0].rearrange("(t p) j -> p t j", p=P))
        xt0 = psm.tile([F_IN, N], f32, name="xtt")
        nc.sync.dma_start(xt0, xt_ap[0])

        # ---- session constants ----
        io_sb = consts.tile([P, N], f32)
        nc.sync.dma_start(io_sb, io_d.ap())
        px_sb = consts.tile([P, T], f32)
        nc.sync.dma_start(px_sb, px_d.ap())
        cv_sb = consts.tile([P, 4], f32)
        nc.sync.dma_start(cv_sb, cv_d.ap())
        bt_sb = consts.tile([P, 2], f32)
        nc.sync.dma_start(bt_sb, bt_d.ap())
        mg_sb = consts.tile([P, NG], u32)
        nc.sync.dma_start(mg_sb, mg_d.ap())
        wr_sb = consts.tile([P, P], f32)
        nc.sync.dma_start(wr_sb, wr_d.ap())
        b1_sb = consts.tile([P, HID], f32)
        nc.sync.dma_start(b1_sb, b1_d.ap())
        pb_sb = consts.tile([P, HID], f32)
        nc.sync.dma_start(pb_sb, pb_d.ap())
        bg_sb = consts.tile([P, HID], f32)
        nc.sync.dma_start(bg_sb, bg_d.ap())
        w1_sb = consts.tile([F_IN, HID], f32)
        nc.sync.dma_start(w1_sb, w1_d.ap())
        gw_sb = consts.tile([HID, HID], f32)
        nc.sync.dma_start(gw_sb, gw_d.ap())
        lw_sb = consts.tile([HID, CLS], f32)
        nc.sync.dma_start(lw_sb, lw_d.ap())
        lb_sb = consts.tile([NG, CLS], f32)
        nc.sync.dma_start(lb_sb, lb_d.ap())

        ones_u = consts.tile([P, NG], u32)
        nc.vector.memset(ones_u, 1)
        ones_col = consts.tile([P, 1], f32)
        nc.vector.memset(ones_col, 1.0)
        onec_bf = consts.tile([P, 1], bf16)
        nc.vector.memset(onec_bf, 1.0)

        eye_sb = consts.tile([P, P], f32)
        make_identity(nc, eye_sb)
        eye_c0 = consts.tile([P, P], f32)
        eye_c1 = consts.tile([P, P], f32)
        eye_c2 = consts.tile([P, P], f32)
        nc.vector.tensor_scalar(out=eye_c0, in0=eye_sb, scalar1=cv_sb[:, 0:1], scalar2=None, op0=Alu.mult)
        nc.vector.tensor_scalar(out=eye_c1, in0=eye_sb, scalar1=cv_sb[:, 1:2], scalar2=None, op0=Alu.mult)
        nc.vector.tensor_scalar(out=eye_c2, in0=eye_sb, scalar1=cv_sb[:, 2:3], scalar2=None, op0=Alu.mult)

        # lower-triangle masks: ltm[p, t, j] = 1 if j < 128*t + p
        ltm = consts.tile([P, T, N], f32)
        for t in range(T):
            nc.gpsimd.tensor_scalar(out=ltm[:, t, :], in0=io_sb, scalar1=px_sb[:, t : t + 1], scalar2=None, op0=Alu.is_lt)

        pooled_all = consts.tile([HID, NG], f32)
        xp_all = consts.tile([P, NG, HID], f32)
        dsel_all = consts.tile([P, NG], f32)
        dgpre_all = consts.tile([P, NG], f32)


        def prefetch(g):
            A = pa.tile([P, T, N], bf16, name="A")
            nc.sync.dma_start(A, adj_ap[g].rearrange("(t p) j -> p t j", p=P))
            xt = psm.tile([F_IN, N], f32, name="xtt")
            nc.sync.dma_start(xt, xt_ap[g])
            return A, xt

        def stage1(g, pre):
            """Filter chains + degree for graph g."""
            A, xt = pre

            # ---- A2 = A@A ; D = c3*A2 + c2*A + c1*I (bf16, exact) ----
            D = pbd.tile([P, T, N], bf16, name="D")
            for i in range(T):
                ps = ppb.tile([P, N], f32, name="psC", tag="big")
                for k in range(T):
                    nc.tensor.matmul(ps, lhsT=A[:, k, _blk(i)], rhs=A[:, k, :], start=(k == 0), stop=(k == T - 1))
                a2t = pwide.tile([P, N], bf16, name="a2t")
                nc.scalar.activation(out=a2t, in_=ps, func=Act.Copy, scale=cv_sb[:, 3:4])
                nc.vector.scalar_tensor_tensor(out=D[:, i, :], in0=A[:, i, :], scalar=cv_sb[:, 2:3], in1=a2t, op0=Alu.mult, op1=Alu.add)
                nc.gpsimd.tensor_tensor(out=D[:, i, _blk(i)], in0=D[:, i, _blk(i)], in1=eye_c1, op=Alu.add)

            # ---- M = A@D + c0*I ; deg row-sums fused into the drains ----
            M = pmm.tile([P, T, N], f32, name="M")
            degr = psm.tile([P, T], f32, name="degr")
            for i in range(T):
                ps = ppb.tile([P, N], f32, name="psM", tag="big")
                for k in range(T):
                    nc.tensor.matmul(ps, lhsT=A[:, k, _blk(i)], rhs=D[:, k, :], start=(k == 0), stop=(k == T - 1))
                if i == 0:
                    nc.vector.tensor_scalar(out=M[:, i, :], in0=ps, scalar1=0.0, scalar2=None, op0=Alu.add, op1=Alu.add, accum_out=degr[:, i : i + 1])
                else:
                    nc.scalar.activation(out=M[:, i, :], in_=ps, func=Act.Copy, accum_out=degr[:, i : i + 1])
                nc.gpsimd.tensor_tensor(out=M[:, i, _blk(i)], in0=M[:, i, _blk(i)], in1=eye_c0, op=Alu.add)

            return dict(A=A, xt=xt, M=M, degr=degr)

        def stage1t(g, st):
            """rsqrt + conv + score for graph g."""
            xt, M, degr = st["xt"], st["M"], st["degr"]
            # deg = clip(degr + c0, 1, inf);  d = deg**-0.5
            deg4 = psm.tile([P, T], f32, name="deg4")
            nc.vector.tensor_scalar(out=deg4, in0=degr, scalar1=cv_sb[:, 0:1], scalar2=1.0, op0=Alu.add, op1=Alu.max)
            d4 = _rsqrt(nc, psm, deg4, mg_sb, ones_u, Alu, f32, u32, name="d4")

            # ---- rhs65 = [ d*(x @ W1) | d ]  per row-chunk ----
            rhs65 = pwide.tile([P, T, HID + 1], f32, name="rhs65")
            for t in range(T):
                psx = pps.tile([P, HID], f32, name="psxw", tag="ps128")
                nc.tensor.matmul(psx, lhsT=xt[:, _blk(t)], rhs=w1_sb, start=True, stop=True)
                nc.vector.tensor_scalar(out=rhs65[:, t, 0:HID], in0=psx, scalar1=d4[:, t : t + 1], scalar2=None, op0=Alu.mult)
                nc.scalar.copy(rhs65[:, t, HID : HID + 1], d4[:, t : t + 1])

            # ---- fused: [h | M@d] = M @ rhs65 ; h = relu(d_i*(.) + b1) ----
            # hsc holds [h | score | d | node_id] as the rhs of the xv gather chain
            hsc = pwide.tile([P, T, HID + 3], f32, name="hsc")
            s1c = psm.tile([P, T], f32, name="s1c")
            s2b = psm.tile([P, T], f32, name="s2b")
            junkh = psm.tile([P, HID], f32, name="junkh")
            for i in range(T):
                ps65 = pp65.tile([P, HID + 1], f32, name="ps65", tag="p65")
                for k in range(T):
                    nc.tensor.matmul(ps65, lhsT=M[:, k, _blk(i)], rhs=rhs65[:, k, :], start=(k == 0), stop=(k == T - 1))
                hraw = psm.tile([P, HID], f32, name="hraw")
                nc.vector.scalar_tensor_tensor(out=hraw, in0=ps65[:, 0:HID], scalar=d4[:, i : i + 1], in1=b1_sb, op0=Alu.mult, op1=Alu.add)
                nc.scalar.activation(out=hsc[:, i, 0:HID], in_=hraw, func=Act.Relu)
                # s1_i = sum_h h*p ;  s2b_i = beta1 * d_i * (M@d)_i
                nc.vector.scalar_tensor_tensor(out=junkh, in0=hsc[:, i, 0:HID], scalar=1.0, in1=pb_sb, op0=Alu.mult, op1=Alu.mult, accum_out=s1c[:, i : i + 1])
                nc.vector.scalar_tensor_tensor(out=s2b[:, i : i + 1], in0=ps65[:, HID : HID + 1], scalar=d4[:, i : i + 1], in1=bt_sb[:, 1:2], op0=Alu.mult, op1=Alu.mult)
                nc.scalar.copy(hsc[:, i, HID + 1 : HID + 2], d4[:, i : i + 1])
                nc.scalar.copy(hsc[:, i, HID + 2 : HID + 3], px_sb[:, i : i + 1])

            # ---- score = tanh(beta0*s1 + s2b) ----
            z4 = psm.tile([P, T], f32, name="z4")
            nc.vector.scalar_tensor_tensor(out=z4, in0=s1c, scalar=bt_sb[:, 0:1], in1=s2b, op0=Alu.mult, op1=Alu.add)
            sc4 = psm.tile([P, T], f32, name="sc4")
            nc.scalar.activation(out=sc4, in_=z4, func=Act.Tanh)
            for i in range(T):
                nc.scalar.copy(hsc[:, i, HID : HID + 1], sc4[:, i : i + 1])

            # ---- broadcast score along free dim via DRAM round trip ----
            nc.sync.dma_start(bass.AP(srow_d, g * N, [[1, P], [P, T]]), sc4)
            scbf = pwide.tile([P, N], f32, name="scb")
            nc.sync.dma_start(scbf, bass.AP(srow_d, g * N, [[0, P], [1, N]]))
            st.update(d4=d4, hsc=hsc, sc4=sc4, scbf=scbf)
            return st

        def stage2a(g, st):
            """Rank/top-k + pooled-feature gather setup for graph g."""
            M, d4, hsc, sc4, scbf = st["M"], st["d4"], st["hsc"], st["sc4"], st["scbf"]
            # ---- exact rank: #(s_j > s_i) + #(s_j == s_i and j < i) ----
            junk2 = pwide.tile([P, N], f32, name="junk2")
            req = psm.tile([P, T], f32, name="req")
            masks = []
            for jc in range(T):
                mj = pwide.tile([P, N], bf16, name="mj%d" % jc)
                eng1 = nc.gpsimd if jc < 2 else nc.vector
                eng1.tensor_scalar(out=mj, in0=scbf, scalar1=sc4[:, jc : jc + 1], scalar2=None, op0=Alu.is_lt)
                masks.append(mj)
            for i in range(T):
                w = P * (i + 1)
                nc.vector.scalar_tensor_tensor(out=junk2[:, 0:w], in0=scbf[:, 0:w], scalar=sc4[:, i : i + 1], in1=ltm[:, i, 0:w], op0=Alu.is_equal, op1=Alu.mult, accum_out=req[:, i : i + 1])
            psRank = pps.tile([P, T], f32, name="psRank", tag="ps128")
            for ic in range(T):
                for jc in range(T):
                    nc.tensor.matmul(psRank[:, ic : ic + 1], lhsT=masks[jc][:, _blk(ic)], rhs=onec_bf, start=(jc == 0), stop=(jc == T - 1))
            rank = psm.tile([P, T], f32, name="rank")
            nc.vector.tensor_tensor(out=rank, in0=psRank, in1=req, op=Alu.add)

            # ---- selection matrix: ST[i,r] = (rank_i == r) ----
            ST = pmm.tile([P, T, K], f32, name="ST")
            for i in range(T):
                nc.vector.tensor_scalar(out=ST[:, i, :], in0=io_sb[:, 0:K], scalar1=rank[:, i : i + 1], scalar2=None, op0=Alu.is_equal)

            # ---- pooled features: [xp0 | vals | dsel | selid] = S @ hsc ----
            psxv = pp65.tile([P, HID + 3], f32, name="psxv", tag="p65")
            for i in range(T):
                nc.tensor.matmul(psxv, lhsT=ST[:, i, :], rhs=hsc[:, i, :], start=(i == 0), stop=(i == T - 1))
            vals = psm.tile([P, 1], f32, name="vals")
            nc.scalar.copy(vals, psxv[:, HID : HID + 1])
            dsel = psm.tile([P, 1], f32, name="dsel")
            nc.scalar.copy(dsel, psxv[:, HID + 1 : HID + 2])
            nc.scalar.copy(dsel_all[:, g : g + 1], psxv[:, HID + 1 : HID + 2])
            selid = psm.tile([P, 1], f32, name="selid")
            nc.scalar.copy(selid, psxv[:, HID + 2 : HID + 3])
            nc.vector.tensor_scalar(out=xp_all[:, g, :], in0=psxv[:, 0:HID], scalar1=vals, scalar2=None, op0=Alu.mult)

            # ---- selected-node index list in wrapped u16 layout via DRAM bcast ----
            nc.sync.dma_start(bass.AP(idx_d, g * P, [[1, P], [1, 1]]), selid)
            selbc = pwide.tile([P, P], f32, name="selbc")
            nc.sync.dma_start(selbc, bass.AP(idx_d, g * P, [[0, P], [1, P]]))
            selmm = pwide.tile([P, P], f32, name="selmm")
            nc.gpsimd.tensor_tensor(out=selmm, in0=selbc, in1=wr_sb, op=Alu.mult)
            idxf = psm.tile([P, 8], f32, name="idxf")
            nc.vector.tensor_reduce(out=idxf, in_=selmm.rearrange("p (s q) -> p s q", q=16), axis=X, op=Alu.max)
            idxw = psm.tile([P, 8], u16, name="idxw")
            nc.vector.tensor_copy(idxw, idxf)

            st.update(ST=ST, idxw=idxw, dsel=dsel, psxv=None)
            return st

        def stage2b(g, st):
            """Pooled adjacency Mp0 + per-graph drains for the batched tail."""
            M, ST, idxw, dsel_c, psxv = st["M"], st["ST"], st["idxw"], st["dsel"], st["psxv"]
            # ---- Mp0[r,r'] = M[sel_r, sel_r'] : free-dim gather + one matmul ----
            G2 = pmm.tile([P, T, K], f32, name="Gsb")
            for i in range(T):
                nc.gpsimd.indirect_copy(out=G2[:, i, :], data=M[:, i, :], idxs=idxw, i_know_ap_gather_is_preferred=True)
            psmp = pps.tile([P, K], f32, name="psmp", tag="ps128")
            for i in range(T):
                nc.tensor.matmul(psmp, lhsT=ST[:, i, :], rhs=G2[:, i, :], start=(i == 0), stop=(i == T - 1))
            Mp0 = pmp.tile([P, K], f32, name="Mp0")
            nc.scalar.copy(Mp0, psmp)
            # dgpre = Mp0 @ dsel
            psdg = pps.tile([P, 1], f32, name="psdg", tag="ps128")
            nc.tensor.matmul(psdg, lhsT=Mp0, rhs=dsel_c, start=True, stop=True)
            nc.scalar.copy(dgpre_all[:, g : g + 1], psdg)
            return Mp0

        def epilogue(mp0s, g0, g1):
            """Batched GCN + readout for graphs [g0, g1)."""
            NB = g1 - g0
            gs = slice(g0, g1)
            # dg = dsel*(Mp0@dsel) + 1 ; di = dg**-0.5   (batched)
            dg_all = psm.tile([P, NB], f32, name="dg_all")
            nc.vector.scalar_tensor_tensor(out=dg_all, in0=dgpre_all[:, gs], scalar=1.0, in1=dsel_all[:, gs], op0=Alu.mult, op1=Alu.mult)
            nc.vector.tensor_scalar(out=dg_all, in0=dg_all, scalar1=1.0, scalar2=None, op0=Alu.add)
            di_all = _rsqrt(nc, psm, dg_all, mg_sb, ones_u, Alu, f32, u32, name="di")
            di_bc = di_all[:, :, None].broadcast_to([P, NB, HID])
            ds_bc = dsel_all[:, gs, None].broadcast_to([P, NB, HID])
            # w = di*xp ; u = dsel*w
            w_all = psm.tile([P, NB, HID], f32, name="w_all")
            nc.vector.tensor_tensor(out=w_all, in0=xp_all[:, gs, :], in1=di_bc, op=Alu.mult)
            u_all = psm.tile([P, NB, HID], f32, name="u_all")
            nc.vector.tensor_tensor(out=u_all, in0=w_all, in1=ds_bc, op=Alu.mult)
            # z = di*(dsel*(Mp0@u) + w) per graph, batched drains
            psz = pp65.tile([P, NB, HID], f32, name="pszall", tag="pbig2", bufs=1)
            for g in range(g0, g1):
                nc.tensor.matmul(psz[:, g - g0, :], lhsT=mp0s[g], rhs=u_all[:, g - g0, :], start=True, stop=True)
            q_all = psm.tile([P, NB, HID], f32, name="q_all")
            nc.vector.tensor_tensor(out=q_all, in0=psz, in1=ds_bc, op=Alu.mult)
            nc.vector.tensor_tensor(out=q_all, in0=q_all, in1=w_all, op=Alu.add)
            g1_all = psm.tile([P, NB, HID], f32, name="g1_all")
            nc.vector.tensor_tensor(out=g1_all, in0=q_all, in1=di_bc, op=Alu.mult)
            # transpose each graph's g1: [128, 64] -> [64, 128]
            g1T_all = psm.tile([HID, NB, P], f32, name="g1T_all")
            for g in range(g0, g1):
                pst_ = pps.tile([HID, P], f32, name="psg1t", tag="ps128")
                nc.tensor.transpose(pst_, g1_all[:, g - g0, :], eye_sb)
                nc.scalar.copy(g1T_all[:, g - g0, :], pst_)
            # h2 = relu(g1 @ gcn_w + gcn_b)
            psh2 = pp65.tile([P, NB, HID], f32, name="psh2all", tag="pbig2", bufs=1)
            for g in range(g0, g1):
                nc.tensor.matmul(psh2[:, g - g0, :], lhsT=g1T_all[:, g - g0, :], rhs=gw_sb, start=True, stop=True)
            bg_bc = bg_sb[:, None, :].broadcast_to([P, NB, HID])
            h2r_all = psm.tile([P, NB, HID], f32, name="h2r_all")
            nc.vector.tensor_tensor(out=h2r_all, in0=psh2, in1=bg_bc, op=Alu.add)
            h2_all = psm.tile([P, NB, HID], f32, name="h2_all")
            nc.scalar.activation(out=h2_all, in_=h2r_all, func=Act.Relu)
            # pooled[c, g] = sum_k h2[k, g, c]
            pspool = pps.tile([HID, NB], f32, name="pspool", tag="ps128")
            for g in range(g0, g1):
                nc.tensor.matmul(pspool[:, g - g0 : g - g0 + 1], lhsT=h2_all[:, g - g0, :], rhs=ones_col, start=True, stop=True)
            nc.scalar.copy(pooled_all[:, gs], pspool)

        stash = {}
        mp0s = {}
        pre = (A0, xt0)
        for g in range(NG):
            stash[g] = stage1(g, pre)
            if g + 1 < NG:
                pre = prefetch(g + 1)
            stash[g] = stage1t(g, stash[g])
            if g >= 1:
                stash[g - 1] = stage2a(g - 1, stash[g - 1])
            if g >= 2:
                mp0s[g - 2] = stage2b(g - 2, stash.pop(g - 2))
        stash[NG - 1] = stage2a(NG - 1, stash[NG - 1])
        mp0s[NG - 2] = stage2b(NG - 2, stash.pop(NG - 2))
        epilogue(mp0s, 0, NG // 2)
        mp0s[NG - 1] = stage2b(NG - 1, stash.pop(NG - 1))
        epilogue(mp0s, NG // 2, NG)

        # ---- head: logits + log_softmax for all graphs at once ----
        pslg = pps.tile([NG, CLS], f32, name="pslg", tag="ps128")
        nc.tensor.matmul(pslg, lhsT=pooled_all, rhs=lw_sb, start=True, stop=True)
        lg = psm.tile([NG, CLS], f32, name="lg")
        nc.vector.tensor_tensor(out=lg, in0=pslg, in1=lb_sb, op=Alu.add)
        mx = psm.tile([NG, 1], f32, name="mx")
        nc.vector.tensor_reduce(out=mx, in_=lg, axis=X, op=Alu.max)
        shv = psm.tile([NG, CLS], f32, name="shv")
        nc.vector.tensor_scalar(out=shv, in0=lg, scalar1=mx, scalar2=None, op0=Alu.subtract)
        ex = psm.tile([NG, CLS], f32, name="ex")
        sm = psm.tile([NG, 1], f32, name="sm")
        nc.scalar.activation(out=ex, in_=shv, func=Act.Exp, accum_out=sm)
        ls = psm.tile([NG, 1], f32, name="ls")
        nc.scalar.activation(out=ls, in_=sm, func=Act.Ln)
        res = psm.tile([NG, CLS], f32, name="res")
        nc.vector.tensor_scalar(out=res, in0=shv, scalar1=ls, scalar2=None, op0=Alu.subtract)
        nc.sync.dma_start(out_d.ap(), res)

    nc.compile()
    return nc


def _get_program():
    if "nc" not in _CACHE:
        _CACHE["nc"] = build_program()
    return _CACHE["nc"]


def make_in_maps(inputs):
    """Host-side prep: shard graphs over cores, broadcast tiny weights."""
    import ml_dtypes

    x = np.asarray(inputs["x"], np.float32)
    adj = np.ascontiguousarray(np.asarray(inputs["adj"], np.float32).astype(ml_dtypes.bfloat16))
    pw = np.asarray(inputs["pan_weight"], np.float32)
    c = np.cumprod(pw).astype(np.float32)  # [c0, c1, c2, c3]
    w1 = np.ascontiguousarray(np.asarray(inputs["conv1_w"], np.float32))
    b1 = np.asarray(inputs["conv1_b"], np.float32)
    pv = np.asarray(inputs["p_vec"], np.float32)
    beta = np.asarray(inputs["beta"], np.float32)
    gw = np.ascontiguousarray(np.asarray(inputs["gcn_w"], np.float32))
    gb = np.asarray(inputs["gcn_b"], np.float32)
    lw = np.ascontiguousarray(np.asarray(inputs["lin_w"], np.float32))
    lb = np.asarray(inputs["lin_b"], np.float32)

    xt = np.ascontiguousarray(x.transpose(0, 2, 1))  # [G, F_IN, N]
    iota = np.tile(np.arange(N, dtype=np.float32), (P, 1))
    pidx = (np.arange(P, dtype=np.float32)[:, None] + P * np.arange(T, dtype=np.float32)[None, :])
    magic = np.full((P, NG), np.uint32(2 * 0x5F3759DF), dtype=np.uint32)
    wrap = (np.arange(P)[None, :] % 16 == np.arange(P)[:, None] % 16).astype(np.float32)

    shared = {
        "w1": w1,
        "gcnw": gw,
        "linw": lw,
        "linb": np.ascontiguousarray(np.tile(lb, (NG, 1))),
        "b1b": np.ascontiguousarray(np.tile(b1, (P, 1))),
        "pb": np.ascontiguousarray(np.tile(pv, (P, 1))),
        "bgb": np.ascontiguousarray(np.tile(gb, (P, 1))),
        "iota": iota,
        "pidx": np.ascontiguousarray(pidx),
        "cvec": np.ascontiguousarray(np.tile(c, (P, 1))),
        "betab": np.ascontiguousarray(np.tile(beta, (P, 1))),
        "magic": magic,
        "wrapidx": np.ascontiguousarray(wrap),
    }
    in_maps = []
    for ci in range(NCORES):
        sl = slice(ci * NG, (ci + 1) * NG)
        m = dict(shared)
        m["adj"] = adj[sl]
        m["xt"] = xt[sl]
        in_maps.append(m)
    return in_maps


def kernel(**inputs):
    from concourse.bass_utils import run_bass_kernel_spmd

    nc = _get_program()
    in_maps = make_in_maps(inputs)
    r = run_bass_kernel_spmd(nc, in_maps, list(range(NCORES)))
    return np.ascontiguousarray(
        np.concatenate([r.results[i]["out"] for i in range(NCORES)], axis=0)
    ).astype(np.float32)



# revision 35
# speedup vs baseline: 1.0255x; 1.0044x over previous
"""Bass/Tile Trainium2 kernel for nn_Net_4698694222696.

PANConv (cubic path-integral filter) + PANPooling (top-k) + GCNConv + sum-pool
+ linear head + log_softmax, data-parallel over the graph dimension:
64 graphs -> 8 NeuronCores x 8 graphs/core (no collectives needed).

Algorithm notes (per graph, N=512 nodes, 4 row-chunks of 128):
  M = c0*I + c1*A + c2*A^2 + c3*A^3 via two bf16 PE chains (exact: A is 0/1
  and D = c3*A^2 + c2*A + c1*I is small-integer*2^-4 for pan_weight=0.5;
  PSUM accumulates fp32): A2 = A@A ; D assembled in the drain ; M = A@D + c0*I.
  Mn = diag(d) M diag(d) never materialized - d folded into matmul operands.
  top-k via exact comparison-counting rank (ties broken by index, matching
  jax.lax.top_k); pooled-feature gather via 0/1 selection matmul; pooled
  adjacency via GPSIMD indirect_copy column gather + one selection matmul.
  rsqrt via uint32 bit-trick seed + 2 Newton iterations on DVE (ACT sqrt is
  imprecise; keeps ACT in one table set so no per-graph table reloads).
  Graph loop is software-pipelined (s1 chains | s1t score | s2a rank | s2b
  gather, lag 2) with a batched GCN epilogue over graph halves.
"""

import numpy as np

G_TOT, N, F_IN, HID, K, CLS = 64, 512, 7, 64, 128, 2
NCORES = 8
NG = G_TOT // NCORES  # graphs per core
P = 128
T = N // P  # 4 row-chunks

_CACHE = {}


def _blk(t):
    return slice(t * P, (t + 1) * P)


def _rsqrt(nc, pool, x, magic_u, ones_u, Alu, f32, u32, name):
    """y = x**-0.5 elementwise for an SBUF tile x of shape [P, w]."""
    w = x.shape[-1]
    yi = pool.tile(list(x.shape), u32, name=name + "_i")
    # seed bits = (2*0x5f3759df - bits(x)) >> 1  ~  0x5f3759df - (bits(x)>>1)
    nc.vector.tensor_tensor(out=yi, in0=magic_u[:, :w], in1=x.bitcast(u32), op=Alu.subtract)
    yi2 = pool.tile(list(x.shape), u32, name=name + "_i2")
    nc.vector.tensor_tensor(out=yi2, in0=yi, in1=ones_u[:, :w], op=Alu.logical_shift_right)
    y = yi2.bitcast(f32)
    t = pool.tile(list(x.shape), f32, name=name + "_t")
    y2 = pool.tile(list(x.shape), f32, name=name + "_y2")
    cur, nxt = y, y2
    for _ in range(2):
        nc.vector.tensor_tensor(out=t, in0=cur, in1=cur, op=Alu.mult)
        nc.vector.tensor_tensor(out=t, in0=t, in1=x, op=Alu.mult)
        nc.vector.tensor_scalar(out=t, in0=t, scalar1=-0.5, scalar2=1.5, op0=Alu.mult, op1=Alu.add)
        nc.vector.tensor_tensor(out=nxt, in0=cur, in1=t, op=Alu.mult)
        cur, nxt = nxt, cur
    return cur


def build_program():
    """Build the single-core SPMD Bass program (same NEFF on all 8 cores)."""
    from contextlib import ExitStack

    import concourse.bass as bass
    import concourse.bacc as bacc
    import concourse.mybir as mybir
    import concourse.tile as tile
    from concourse.masks import make_identity

    f32 = mybir.dt.float32
    bf16 = mybir.dt.bfloat16
    u32 = mybir.dt.uint32
    u16 = mybir.dt.uint16
    Alu = mybir.AluOpType
    Act = mybir.ActivationFunctionType
    X = mybir.AxisListType.X

    nc = bacc.Bacc("TRN2", target_bir_lowering=False, debug=False, num_devices=NCORES)

    # ---- per-core DRAM I/O ----
    adj_d = nc.dram_tensor("adj", [NG, N, N], bf16, kind="ExternalInput")
    xt_d = nc.dram_tensor("xt", [NG, F_IN, N], f32, kind="ExternalInput")  # x^T per graph
    w1_d = nc.dram_tensor("w1", [F_IN, HID], f32, kind="ExternalInput")
    gw_d = nc.dram_tensor("gcnw", [HID, HID], f32, kind="ExternalInput")
    lw_d = nc.dram_tensor("linw", [HID, CLS], f32, kind="ExternalInput")
    lb_d = nc.dram_tensor("linb", [NG, CLS], f32, kind="ExternalInput")
    b1_d = nc.dram_tensor("b1b", [P, HID], f32, kind="ExternalInput")
    pb_d = nc.dram_tensor("pb", [P, HID], f32, kind="ExternalInput")
    bg_d = nc.dram_tensor("bgb", [P, HID], f32, kind="ExternalInput")
    io_d = nc.dram_tensor("iota", [P, N], f32, kind="ExternalInput")
    px_d = nc.dram_tensor("pidx", [P, T], f32, kind="ExternalInput")
    cv_d = nc.dram_tensor("cvec", [P, 4], f32, kind="ExternalInput")
    bt_d = nc.dram_tensor("betab", [P, 2], f32, kind="ExternalInput")
    mg_d = nc.dram_tensor("magic", [P, NG], u32, kind="ExternalInput")
    wr_d = nc.dram_tensor("wrapidx", [P, P], f32, kind="ExternalInput")
    out_d = nc.dram_tensor("out", [NG, CLS], f32, kind="ExternalOutput")
    # internal DRAM scratch for the score row-broadcast round trip
    srow_d = nc.dram_tensor("srow", [NG, N], f32)
    idx_d = nc.dram_tensor("idxscr", [NG, P], f32)

    adj_ap = adj_d.ap()
    xt_ap = xt_d.ap()

    with tile.TileContext(nc) as tc, ExitStack() as ctx:
        consts = ctx.enter_context(tc.tile_pool(name="consts", bufs=1))
        pa = ctx.enter_context(tc.tile_pool(name="pa", bufs=3))
        pbd = ctx.enter_context(tc.tile_pool(name="pbd", bufs=2))
        pmm = ctx.enter_context(tc.tile_pool(name="pmm", bufs=3))
        psm = ctx.enter_context(tc.tile_pool(name="psm", bufs=2))
        pwide = ctx.enter_context(tc.tile_pool(name="pwide", bufs=2))
        ppb = ctx.enter_context(tc.tile_pool(name="ppb", bufs=2, space="PSUM"))
        pp65 = ctx.enter_context(tc.tile_pool(name="pp65", bufs=2, space="PSUM"))
        pps = ctx.enter_context(tc.tile_pool(name="pps", bufs=3, space="PSUM"))
        pmp = ctx.enter_context(tc.tile_pool(name="pmp", bufs=NG))

        # ---- prefetch graph 0 before the constant DMAs ----
        A0 = pa.tile([P, T, N], bf16, name="A")
        nc.sync.dma_start(A0, adj_ap[0].rearrange("(t p) j -> p t j", p=P))
        xt0 = psm.tile([F_IN, N], f32, name="xtt")
        nc.sync.dma_start(xt0, xt_ap[0])

        # ---- session constants ----
        io_sb = consts.tile([P, N], f32)
        nc.sync.dma_start(io_sb, io_d.ap())
        px_sb = consts.tile([P, T], f32)
        nc.sync.dma_start(px_sb, px_d.ap())
        cv_sb = consts.tile([P, 4], f32)
        nc.sync.dma_start(cv_sb, cv_d.ap())
        bt_sb = consts.tile([P, 2], f32)
        nc.sync.dma_start(bt_sb, bt_d.ap())
        mg_sb = consts.tile([P, NG], u32)
        nc.sync.dma_start(mg_sb, mg_d.ap())
        wr_sb = consts.tile([P, P], f32)
        nc.sync.dma_start(wr_sb, wr_d.ap())
        b1_sb = consts.tile([P, HID], f32)
        nc.sync.dma_start(b1_sb, b1_d.ap())
        pb_sb = consts.tile([P, HID], f32)
        nc.sync.dma_start(pb_sb, pb_d.ap())
        bg_sb = consts.tile([P, HID], f32)
        nc.sync.dma_start(bg_sb, bg_d.ap())
        w1_sb = consts.tile([F_IN, HID], f32)
        nc.sync.dma_start(w1_sb, w1_d.ap())
        gw_sb = consts.tile([HID, HID], f32)
        nc.sync.dma_start(gw_sb, gw_d.ap())
        lw_sb = consts.tile([HID, CLS], f32)
        nc.sync.dma_start(lw_sb, lw_d.ap())
        lb_sb = consts.tile([NG, CLS], f32)
        nc.sync.dma_start(lb_sb, lb_d.ap())

        ones_u = consts.tile([P, NG], u32)
        nc.vector.memset(ones_u, 1)
        ones_col = consts.tile([P, 1], f32)
        nc.vector.memset(ones_col, 1.0)
        onec_bf = consts.tile([P, 1], bf16)
        nc.vector.memset(onec_bf, 1.0)

        eye_sb = consts.tile([P, P], f32)
        make_identity(nc, eye_sb)
        eye_c0 = consts.tile([P, P], f32)
        eye_c1 = consts.tile([P, P], f32)
        eye_c2 = consts.tile([P, P], f32)
        nc.vector.tensor_scalar(out=eye_c0, in0=eye_sb, scalar1=cv_sb[:, 0:1], scalar2=None, op0=Alu.mult)
        nc.vector.tensor_scalar(out=eye_c1, in0=eye_sb, scalar1=cv_sb[:, 1:2], scalar2=None, op0=Alu.mult)
        nc.vector.tensor_scalar(out=eye_c2, in0=eye_sb, scalar1=cv_sb[:, 2:3], scalar2=None, op0=Alu.mult)

        # lower-triangle masks: ltm[p, t, j] = 1 if j < 128*t + p
        ltm = consts.tile([P, T, N], f32)
        for t in range(T):
            nc.gpsimd.tensor_scalar(out=ltm[:, t, :], in0=io_sb, scalar1=px_sb[:, t : t + 1], scalar2=None, op0=Alu.is_lt)

        pooled_all = consts.tile([HID, NG], f32)
        xp_all = consts.tile([P, NG, HID], f32)
        dsel_all = consts.tile([P, NG], f32)
        dgpre_all = consts.tile([P, NG], f32)


        def prefetch(g):
            A = pa.tile([P, T, N], bf16, name="A")
            nc.sync.dma_start(A, adj_ap[g].rearrange("(t p) j -> p t j", p=P))
            xt = psm.tile([F_IN, N], f32, name="xtt")
            nc.sync.dma_start(xt, xt_ap[g])
            return A, xt

        def stage1(g, pre):
            """Filter chains + degree for graph g."""
            A, xt = pre

            # ---- A2 = A@A ; D = c3*A2 + c2*A + c1*I (bf16, exact) ----
            D = pbd.tile([P, T, N], bf16, name="D")
            for i in range(T):
                ps = ppb.tile([P, N], f32, name="psC", tag="big")
                for k in range(T):
                    nc.tensor.matmul(ps, lhsT=A[:, k, _blk(i)], rhs=A[:, k, :], start=(k == 0), stop=(k == T - 1))
                a2t = pwide.tile([P, N], bf16, name="a2t")
                nc.scalar.activation(out=a2t, in_=ps, func=Act.Copy, scale=cv_sb[:, 3:4])
                nc.vector.scalar_tensor_tensor(out=D[:, i, :], in0=A[:, i, :], scalar=cv_sb[:, 2:3], in1=a2t, op0=Alu.mult, op1=Alu.add)
                nc.gpsimd.tensor_tensor(out=D[:, i, _blk(i)], in0=D[:, i, _blk(i)], in1=eye_c1, op=Alu.add)

            # ---- M = A@D + c0*I ; deg row-sums fused into the drains ----
            M = pmm.tile([P, T, N], f32, name="M")
            degr = psm.tile([P, T], f32, name="degr")
            for i in range(T):
                ps = ppb.tile([P, N], f32, name="psM", tag="big")
                for k in range(T):
                    nc.tensor.matmul(ps, lhsT=A[:, k, _blk(i)], rhs=D[:, k, :], start=(k == 0), stop=(k == T - 1))
                if i == 0:
                    nc.vector.tensor_scalar(out=M[:, i, :], in0=ps, scalar1=0.0, scalar2=None, op0=Alu.add, op1=Alu.add, accum_out=degr[:, i : i + 1])
                else:
                    nc.scalar.activation(out=M[:, i, :], in_=ps, func=Act.Copy, accum_out=degr[:, i : i + 1])
                nc.gpsimd.tensor_tensor(out=M[:, i, _blk(i)], in0=M[:, i, _blk(i)], in1=eye_c0, op=Alu.add)

            return dict(A=A, xt=xt, M=M, degr=degr)

        def stage1t(g, st):
            """rsqrt + conv + score for graph g."""
            xt, M, degr = st["xt"], st["M"], st["degr"]
            # deg = clip(degr + c0, 1, inf);  d = deg**-0.5
            deg4 = psm.tile([P, T], f32, name="deg4")
            nc.vector.tensor_scalar(out=deg4, in0=degr, scalar1=cv_sb[:, 0:1], scalar2=1.0, op0=Alu.add, op1=Alu.max)
            d4 = _rsqrt(nc, psm, deg4, mg_sb, ones_u, Alu, f32, u32, name="d4")

            # ---- rhs65 = [ d*(x @ W1) | d ]  per row-chunk ----
            rhs65 = pwide.tile([P, T, HID + 1], f32, name="rhs65")
            for t in range(T):
                psx = pps.tile([P, HID], f32, name="psxw", tag="ps128")
                nc.tensor.matmul(psx, lhsT=xt[:, _blk(t)], rhs=w1_sb, start=True, stop=True)
                nc.vector.tensor_scalar(out=rhs65[:, t, 0:HID], in0=psx, scalar1=d4[:, t : t + 1], scalar2=None, op0=Alu.mult)
                nc.scalar.copy(rhs65[:, t, HID : HID + 1], d4[:, t : t + 1])

            # ---- fused: [h | M@d] = M @ rhs65 ; h = relu(d_i*(.) + b1) ----
            # hsc holds [h | score | d | node_id] as the rhs of the xv gather chain
            hsc = pwide.tile([P, T, HID + 3], f32, name="hsc")
            s1c = psm.tile([P, T], f32, name="s1c")
            s2b = psm.tile([P, T], f32, name="s2b")
            junkh = psm.tile([P, HID], f32, name="junkh")
            for i in range(T):
                ps65 = pp65.tile([P, HID + 1], f32, name="ps65", tag="p65")
                for k in range(T):
                    nc.tensor.matmul(ps65, lhsT=M[:, k, _blk(i)], rhs=rhs65[:, k, :], start=(k == 0), stop=(k == T - 1))
                hraw = psm.tile([P, HID], f32, name="hraw")
                nc.vector.scalar_tensor_tensor(out=hraw, in0=ps65[:, 0:HID], scalar=d4[:, i : i + 1], in1=b1_sb, op0=Alu.mult, op1=Alu.add)
                nc.scalar.activation(out=hsc[:, i, 0:HID], in_=hraw, func=Act.Relu)
                # s1_i = sum_h h*p ;  s2b_i = beta1 * d_i * (M@d)_i
                nc.vector.scalar_tensor_tensor(out=junkh, in0=hsc[:, i, 0:HID], scalar=1.0, in1=pb_sb, op0=Alu.mult, op1=Alu.mult, accum_out=s1c[:, i : i + 1])
                nc.vector.scalar_tensor_tensor(out=s2b[:, i : i + 1], in0=ps65[:, HID : HID + 1], scalar=d4[:, i : i + 1], in1=bt_sb[:, 1:2], op0=Alu.mult, op1=Alu.mult)
                nc.scalar.copy(hsc[:, i, HID + 1 : HID + 2], d4[:, i : i + 1])
                nc.scalar.copy(hsc[:, i, HID + 2 : HID + 3], px_sb[:, i : i + 1])

            # ---- score = tanh(beta0*s1 + s2b) ----
            z4 = psm.tile([P, T], f32, name="z4")
            nc.vector.scalar_tensor_tensor(out=z4, in0=s1c, scalar=bt_sb[:, 0:1], in1=s2b, op0=Alu.mult, op1=Alu.add)
            sc4 = psm.tile([P, T], f32, name="sc4")
            nc.scalar.activation(out=sc4, in_=z4, func=Act.Tanh)
            for i in range(T):
                nc.scalar.copy(hsc[:, i, HID : HID + 1], sc4[:, i : i + 1])

            # ---- broadcast score along free dim via DRAM round trip ----
            nc.sync.dma_start(bass.AP(srow_d, g * N, [[1, P], [P, T]]), sc4)
            scbf = pwide.tile([P, N], f32, name="scb")
            nc.sync.dma_start(scbf, bass.AP(srow_d, g * N, [[0, P], [1, N]]))
            st.update(d4=d4, hsc=hsc, sc4=sc4, scbf=scbf)
            return st

        def stage2a(g, st):
            """Rank/top-k + pooled-feature gather setup for graph g."""
            M, d4, hsc, sc4, scbf = st["M"], st["d4"], st["hsc"], st["sc4"], st["scbf"]
            # ---- exact rank: #(s_j > s_i) + #(s_j == s_i and j < i) ----
            junk2 = pwide.tile([P, N], f32, name="junk2")
            req = psm.tile([P, T], f32, name="req")
            masks = []
            for jc in range(T):
                mj = pwide.tile([P, N], bf16, name="mj%d" % jc)
                eng1 = nc.gpsimd if jc < 2 else nc.vector
                eng1.tensor_scalar(out=mj, in0=scbf, scalar1=sc4[:, jc : jc + 1], scalar2=None, op0=Alu.is_lt)
                masks.append(mj)
            for i in range(T):
                w = P * (i + 1)
                nc.vector.scalar_tensor_tensor(out=junk2[:, 0:w], in0=scbf[:, 0:w], scalar=sc4[:, i : i + 1], in1=ltm[:, i, 0:w], op0=Alu.is_equal, op1=Alu.mult, accum_out=req[:, i : i + 1])
            psRank = pps.tile([P, T], f32, name="psRank", tag="ps128")
            for ic in range(T):
                for jc in range(T):
                    nc.tensor.matmul(psRank[:, ic : ic + 1], lhsT=masks[jc][:, _blk(ic)], rhs=onec_bf, start=(jc == 0), stop=(jc == T - 1))
            rank = psm.tile([P, T], f32, name="rank")
            nc.vector.tensor_tensor(out=rank, in0=psRank, in1=req, op=Alu.add)

            # ---- selection matrix: ST[i,r] = (rank_i == r) ----
            ST = pmm.tile([P, T, K], f32, name="ST")
            for i in range(T):
                nc.vector.tensor_scalar(out=ST[:, i, :], in0=io_sb[:, 0:K], scalar1=rank[:, i : i + 1], scalar2=None, op0=Alu.is_equal)

            # ---- pooled features: [xp0 | vals | dsel | selid] = S @ hsc ----
            psxv = pp65.tile([P, HID + 3], f32, name="psxv", tag="p65")
            for i in range(T):
                nc.tensor.matmul(psxv, lhsT=ST[:, i, :], rhs=hsc[:, i, :], start=(i == 0), stop=(i == T - 1))
            vals = psm.tile([P, 1], f32, name="vals")
            nc.scalar.copy(vals, psxv[:, HID : HID + 1])
            dsel = psm.tile([P, 1], f32, name="dsel")
            nc.scalar.copy(dsel, psxv[:, HID + 1 : HID + 2])
            nc.scalar.copy(dsel_all[:, g : g + 1], psxv[:, HID + 1 : HID + 2])
            selid = psm.tile([P, 1], f32, name="selid")
            nc.scalar.copy(selid, psxv[:, HID + 2 : HID + 3])
            nc.vector.tensor_scalar(out=xp_all[:, g, :], in0=psxv[:, 0:HID], scalar1=vals, scalar2=None, op0=Alu.mult)

            # ---- selected-node index list in wrapped u16 layout via DRAM bcast ----
            nc.sync.dma_start(bass.AP(idx_d, g * P, [[1, P], [1, 1]]), selid)
            selbc = pwide.tile([P, P], f32, name="selbc")
            nc.sync.dma_start(selbc, bass.AP(idx_d, g * P, [[0, P], [1, P]]))
            selmm = pwide.tile([P, P], f32, name="selmm")
            nc.gpsimd.tensor_tensor(out=selmm, in0=selbc, in1=wr_sb, op=Alu.mult)
            idxf = psm.tile([P, 8], f32, name="idxf")
            nc.vector.tensor_reduce(out=idxf, in_=selmm.rearrange("p (s q) -> p s q", q=16), axis=X, op=Alu.max)
            idxw = psm.tile([P, 8], u16, name="idxw")
            nc.vector.tensor_copy(idxw, idxf)

            st.update(ST=ST, idxw=idxw, dsel=dsel, psxv=None)
            return st

        def stage2b(g, st):
            """Pooled adjacency Mp0 + per-graph drains for the batched tail."""
            M, ST, idxw, dsel_c, psxv = st["M"], st["ST"], st["idxw"], st["dsel"], st["psxv"]
            # ---- Mp0[r,r'] = M[sel_r, sel_r'] : free-dim gather + one matmul ----
            G2 = pmm.tile([P, T, K], f32, name="Gsb")
            for i in range(T):
                nc.gpsimd.indirect_copy(out=G2[:, i, :], data=M[:, i, :], idxs=idxw, i_know_ap_gather_is_preferred=True)
            psmp = pps.tile([P, K], f32, name="psmp", tag="ps128")
            for i in range(T):
                nc.tensor.matmul(psmp, lhsT=ST[:, i, :], rhs=G2[:, i, :], start=(i == 0), stop=(i == T - 1))
            Mp0 = pmp.tile([P, K], f32, name="Mp0")
            nc.scalar.copy(Mp0, psmp)
            # dgpre = Mp0 @ dsel
            psdg = pps.tile([P, 1], f32, name="psdg", tag="ps128")
            nc.tensor.matmul(psdg, lhsT=Mp0, rhs=dsel_c, start=True, stop=True)
            nc.scalar.copy(dgpre_all[:, g : g + 1], psdg)
            return Mp0

        def epilogue(mp0s, g0, g1):
            """Batched GCN + readout for graphs [g0, g1)."""
            NB = g1 - g0
            gs = slice(g0, g1)
            # dg = dsel*(Mp0@dsel) + 1 ; di = dg**-0.5   (batched)
            dg_all = psm.tile([P, NB], f32, name="dg_all")
            nc.vector.scalar_tensor_tensor(out=dg_all, in0=dgpre_all[:, gs], scalar=1.0, in1=dsel_all[:, gs], op0=Alu.mult, op1=Alu.mult)
            nc.vector.tensor_scalar(out=dg_all, in0=dg_all, scalar1=1.0, scalar2=None, op0=Alu.add)
            di_all = _rsqrt(nc, psm, dg_all, mg_sb, ones_u, Alu, f32, u32, name="di")
            # ed = di*dsel, fq = di*di; u = xp*ed; g1 = psz*ed + xp*fq
            ed = psm.tile([P, NB], f32, name="ed")
            nc.vector.tensor_tensor(out=ed, in0=di_all, in1=dsel_all[:, gs], op=Alu.mult)
            fq = psm.tile([P, NB], f32, name="fq")
            nc.vector.tensor_tensor(out=fq, in0=di_all, in1=di_all, op=Alu.mult)
            ed_bc = ed[:, :, None].broadcast_to([P, NB, HID])
            fq_bc = fq[:, :, None].broadcast_to([P, NB, HID])
            u_all = psm.tile([P, NB, HID], f32, name="u_all")
            nc.vector.tensor_tensor(out=u_all, in0=xp_all[:, gs, :], in1=ed_bc, op=Alu.mult)
            psz = pp65.tile([P, NB, HID], f32, name="pszall", tag="pbig2", bufs=1)
            for g in range(g0, g1):
                nc.tensor.matmul(psz[:, g - g0, :], lhsT=mp0s[g], rhs=u_all[:, g - g0, :], start=True, stop=True)
            q_all = psm.tile([P, NB, HID], f32, name="q_all")
            nc.vector.tensor_tensor(out=q_all, in0=psz, in1=ed_bc, op=Alu.mult)
            g1_all = psm.tile([P, NB, HID], f32, name="g1_all")
            nc.vector.scalar_tensor_tensor(out=g1_all, in0=xp_all[:, gs, :], scalar=1.0, in1=fq_bc, op0=Alu.mult, op1=Alu.mult)
            nc.vector.tensor_tensor(out=g1_all, in0=g1_all, in1=q_all, op=Alu.add)
            # transpose each graph's g1: [128, 64] -> [64, 128]
            g1T_all = psm.tile([HID, NB, P], f32, name="g1T_all")
            for g in range(g0, g1):
                pst_ = pps.tile([HID, P], f32, name="psg1t", tag="ps128")
                nc.tensor.transpose(pst_, g1_all[:, g - g0, :], eye_sb)
                nc.scalar.copy(g1T_all[:, g - g0, :], pst_)
            # h2 = relu(g1 @ gcn_w + gcn_b)
            psh2 = pp65.tile([P, NB, HID], f32, name="psh2all", tag="pbig2", bufs=1)
            for g in range(g0, g1):
                nc.tensor.matmul(psh2[:, g - g0, :], lhsT=g1T_all[:, g - g0, :], rhs=gw_sb, start=True, stop=True)
            bg_bc = bg_sb[:, None, :].broadcast_to([P, NB, HID])
            h2r_all = psm.tile([P, NB, HID], f32, name="h2r_all")
            nc.vector.tensor_tensor(out=h2r_all, in0=psh2, in1=bg_bc, op=Alu.add)
            h2_all = psm.tile([P, NB, HID], f32, name="h2_all")
            nc.scalar.activation(out=h2_all, in_=h2r_all, func=Act.Relu)
            # pooled[c, g] = sum_k h2[k, g, c]
            pspool = pps.tile([HID, NB], f32, name="pspool", tag="ps128")
            for g in range(g0, g1):
                nc.tensor.matmul(pspool[:, g - g0 : g - g0 + 1], lhsT=h2_all[:, g - g0, :], rhs=ones_col, start=True, stop=True)
            nc.scalar.copy(pooled_all[:, gs], pspool)

        stash = {}
        mp0s = {}
        pre = (A0, xt0)
        for g in range(NG):
            stash[g] = stage1(g, pre)
            if g + 1 < NG:
                pre = prefetch(g + 1)
            stash[g] = stage1t(g, stash[g])
            if g >= 1:
                stash[g - 1] = stage2a(g - 1, stash[g - 1])
            if g >= 2:
                mp0s[g - 2] = stage2b(g - 2, stash.pop(g - 2))
        stash[NG - 1] = stage2a(NG - 1, stash[NG - 1])
        mp0s[NG - 2] = stage2b(NG - 2, stash.pop(NG - 2))
        epilogue(mp0s, 0, NG // 2)
        mp0s[NG - 1] = stage2b(NG - 1, stash.pop(NG - 1))
        epilogue(mp0s, NG // 2, NG)

        # ---- head: logits + log_softmax for all graphs at once ----
        pslg = pps.tile([NG, CLS], f32, name="pslg", tag="ps128")
        nc.tensor.matmul(pslg, lhsT=pooled_all, rhs=lw_sb, start=True, stop=True)
        lg = psm.tile([NG, CLS], f32, name="lg")
        nc.vector.tensor_tensor(out=lg, in0=pslg, in1=lb_sb, op=Alu.add)
        mx = psm.tile([NG, 1], f32, name="mx")
        nc.vector.tensor_reduce(out=mx, in_=lg, axis=X, op=Alu.max)
        shv = psm.tile([NG, CLS], f32, name="shv")
        nc.vector.tensor_scalar(out=shv, in0=lg, scalar1=mx, scalar2=None, op0=Alu.subtract)
        ex = psm.tile([NG, CLS], f32, name="ex")
        sm = psm.tile([NG, 1], f32, name="sm")
        nc.scalar.activation(out=ex, in_=shv, func=Act.Exp, accum_out=sm)
        ls = psm.tile([NG, 1], f32, name="ls")
        nc.scalar.activation(out=ls, in_=sm, func=Act.Ln)
        res = psm.tile([NG, CLS], f32, name="res")
        nc.vector.tensor_scalar(out=res, in0=shv, scalar1=ls, scalar2=None, op0=Alu.subtract)
        nc.sync.dma_start(out_d.ap(), res)

    nc.compile()
    return nc


def _get_program():
    if "nc" not in _CACHE:
        _CACHE["nc"] = build_program()
    return _CACHE["nc"]


def make_in_maps(inputs):
    """Host-side prep: shard graphs over cores, broadcast tiny weights."""
    import ml_dtypes

    x = np.asarray(inputs["x"], np.float32)
    adj = np.ascontiguousarray(np.asarray(inputs["adj"], np.float32).astype(ml_dtypes.bfloat16))
    pw = np.asarray(inputs["pan_weight"], np.float32)
    c = np.cumprod(pw).astype(np.float32)  # [c0, c1, c2, c3]
    w1 = np.ascontiguousarray(np.asarray(inputs["conv1_w"], np.float32))
    b1 = np.asarray(inputs["conv1_b"], np.float32)
    pv = np.asarray(inputs["p_vec"], np.float32)
    beta = np.asarray(inputs["beta"], np.float32)
    gw = np.ascontiguousarray(np.asarray(inputs["gcn_w"], np.float32))
    gb = np.asarray(inputs["gcn_b"], np.float32)
    lw = np.ascontiguousarray(np.asarray(inputs["lin_w"], np.float32))
    lb = np.asarray(inputs["lin_b"], np.float32)

    xt = np.ascontiguousarray(x.transpose(0, 2, 1))  # [G, F_IN, N]
    iota = np.tile(np.arange(N, dtype=np.float32), (P, 1))
    pidx = (np.arange(P, dtype=np.float32)[:, None] + P * np.arange(T, dtype=np.float32)[None, :])
    magic = np.full((P, NG), np.uint32(2 * 0x5F3759DF), dtype=np.uint32)
    wrap = (np.arange(P)[None, :] % 16 == np.arange(P)[:, None] % 16).astype(np.float32)

    shared = {
        "w1": w1,
        "gcnw": gw,
        "linw": lw,
        "linb": np.ascontiguousarray(np.tile(lb, (NG, 1))),
        "b1b": np.ascontiguousarray(np.tile(b1, (P, 1))),
        "pb": np.ascontiguousarray(np.tile(pv, (P, 1))),
        "bgb": np.ascontiguousarray(np.tile(gb, (P, 1))),
        "iota": iota,
        "pidx": np.ascontiguousarray(pidx),
        "cvec": np.ascontiguousarray(np.tile(c, (P, 1))),
        "betab": np.ascontiguousarray(np.tile(beta, (P, 1))),
        "magic": magic,
        "wrapidx": np.ascontiguousarray(wrap),
    }
    in_maps = []
    for ci in range(NCORES):
        sl = slice(ci * NG, (ci + 1) * NG)
        m = dict(shared)
        m["adj"] = adj[sl]
        m["xt"] = xt[sl]
        in_maps.append(m)
    return in_maps


def kernel(**inputs):
    from concourse.bass_utils import run_bass_kernel_spmd

    nc = _get_program()
    in_maps = make_in_maps(inputs)
    r = run_bass_kernel_spmd(nc, in_maps, list(range(NCORES)))
    return np.ascontiguousarray(
        np.concatenate([r.results[i]["out"] for i in range(NCORES)], axis=0)
    ).astype(np.float32)

